# revision 34
# baseline (speedup 1.0000x reference)
"""BEiT-style attention (B=64, N=197, C=768, H=12, rel-pos bias) on 8 TRN2 cores.

Data-parallel over batch: 8 batch items per core, no collectives.

Design notes (what made this fast — 232us baseline -> ~166us):
  - Engines execute their queues IN ORDER, so emission order IS the
    schedule, and the TRN2 PE has p-states (full 2.4 GHz only after ~3us
    of continuous execution; a stall drops it to 1.2 GHz). The whole
    kernel is one software-pipelined stream in which the tensor engine
    never head-blocks: per head-pair p, S/exp runs two b-iterations
    ahead of O/denom, and the qk GEMM of pair p+1 (proj chunks during
    p=5) is emitted block-wise between them as tensor filler that covers
    the exp->mul->O latency.
  - Rel-pos bias enters as a multiplicative exp(bias) table applied to
    exp(S) on the DVE (2-byte 2x mode) instead of an identity-matmul
    PSUM prefill; softmax denominators use ones-matmuls batched over
    b-pairs, and the ~5x-faster reciprocal_approx_fast custom DVE op.
  - All attention matmuls use K=128 (token chunks padded cross-batch;
    the exp(bias) table zeroes the pad rows so they contribute nothing).
  - Projection runs over 13 flat 128-token chunks (MPAD = 1664 = 13*128);
    proj bias is added by the DVE during PSUM evacuation (no ones-row
    matmul). v_bias is folded into the proj bias on the host (softmax
    rows sum to 1); q_bias/scale fold into the qk weights / ACT evac.
  - Every DRAM input is host-prearranged to its exact SBUF layout; the
    wv/x loads are split so the first v matmul starts ~5us earlier, and
    ~23 warmup matmuls on dummy data keep the PE busy (and its p-state
    ramped) during the input-DMA wait — the v stage then starts at full
    clock with all inputs resident.
    (Empirically, finer DMA splitting [3+ pieces of a hot tile] or
    fine-grained interleaving of accumulation groups makes ALL matmuls
    ~20% slower — keep hot tiles to <=2 DMA pieces and keep matmul
    accumulation groups contiguous.)
  - fp8 e4m3 DoubleRow matmuls (2 fp8/partition/cycle, two 128-deep
    k-tiles per instruction) halve the q/k GEMM and the softmax
    denominator matmuls. q/k evacuate to bf16 (fp8 storage would blow
    the error budget: 2.1e-2 > 2e-2); weights carry a 64x scale so the
    fp8 values sit in e4m3's normal range, undone in the exp evac
    (scale=1/4096). Denominators read an fp8 cast of E (one extra DVE
    copy per (p,b)); the quantization averages out over 197 summands.
    v/proj/O GEMMs must stay bf16 — their error hits the output 1:1,
    and two-sided fp8 costs ~2.6%: over budget. Expected ~1.66e-2.
"""

import numpy as np
import ml_dtypes

import concourse.bass as bass
import concourse.mybir as mybir
import concourse.tile as tile
from concourse import bacc
from concourse.bass_utils import run_bass_kernel_spmd

BF16 = ml_dtypes.bfloat16
E4M3 = ml_dtypes.float8_e4m3
F32 = mybir.dt.float32
BF = mybir.dt.bfloat16
F8 = mybir.dt.float8e4
DR = mybir.MatmulPerfMode.DoubleRow

B, N, C = 64, 197, 768
H, HD = 12, 64
NCORES = 8
BL = B // NCORES            # 8 batches per core
M = BL * N                  # 1576 real tokens per core
MPAD = 1664                 # 13 * 128
SCALE = HD ** -0.5
MCH = [(0, 512), (512, 512), (1024, 512), (1536, 128)]
# two equal f-halves so the (v/proj) PSUM pair evacuates as ONE
# rectangular [2, 384] op — the side engines are op-count-bound
# (~165-400ns fixed cost per instruction regardless of size)
FCH = [(0, 384), (384, 384)]

_NC = None


def _build():
    nc = bacc.Bacc("TRN2", target_bir_lowering=False, debug=False)

    x = nc.dram_tensor("x", [128, 6, MPAD], BF, kind="ExternalInput")
    x8 = nc.dram_tensor("x8", [128, 6, MPAD], F8, kind="ExternalInput")
    wqk = nc.dram_tensor("wqk", [128, 12 * C], F8, kind="ExternalInput")
    wv = nc.dram_tensor("wv", [128, 6, C], BF, kind="ExternalInput")
    wp = nc.dram_tensor("wp", [128, 6 * C], BF, kind="ExternalInput")
    eb = nc.dram_tensor("eb", [128, 12 * 512], BF, kind="ExternalInput")
    qb = nc.dram_tensor("qb", [128, 6], F32, kind="ExternalInput")
    pbb = nc.dram_tensor("pbb", [128, C], BF, kind="ExternalInput")
    out = nc.dram_tensor("out", [M, C], BF, kind="ExternalOutput")

    Ident = mybir.ActivationFunctionType.Identity
    Copy = mybir.ActivationFunctionType.Copy
    Exp = mybir.ActivationFunctionType.Exp

    with tile.TileContext(nc) as tc:
        with (
            tc.tile_pool(name="persist", bufs=1) as P,
            tc.tile_pool(name="et", bufs=4) as et_pool,
            tc.tile_pool(name="e8", bufs=4) as e8_pool,
            tc.tile_pool(name="rcp", bufs=4) as r_pool,
            tc.tile_pool(name="ob", bufs=3) as ob_pool,
            # PSUM = 16KB/partition = 8 banks. One ring of 3 two-bank
            # (4KB) tiles for qk/v/proj/psH2/psD + one ring of 2 one-bank
            # tiles for psOD/warmups: 3*4 + 2*2 = 16KB exactly.
            tc.tile_pool(name="mm2", bufs=3, space="PSUM") as mm2,
            tc.tile_pool(name="mmo", bufs=2, space="PSUM") as mmo,
        ):
            # ---- inputs to SBUF (one DMA per tensor, pre-laid-out) ----
            # wv/x split by first use; more than 2 pieces per hot tile
            # measurably slows every matmul down (see module docstring).
            wv_sb = P.tile([128, 6, C], BF, tag="wv")
            xT = P.tile([128, 6, MPAD], BF, tag="xt")
            # staged by first use: the first v matmuls (batch 0, f-half 0)
            # need only wv cols 0:512 and x tokens 0:256; the 640 split
            # point buys ~9.6us of v compute before the x tail is needed
            # fused v evac needs both f-halves at the first group, so wv
            # loads in one piece before the x head
            nc.sync.dma_start(wv_sb[:, :, :], wv[:, :, :])
            nc.sync.dma_start(xT[:, :, 0:640], x[:, :, 0:640])
            nc.sync.dma_start(xT[:, :, 640:MPAD], x[:, :, 640:MPAD])
            x8_sb = P.tile([128, 6, MPAD], F8, tag="x8")
            nc.sync.dma_start(x8_sb[:, :, :], x8[:, :, :])
            wqk_sb = P.tile([128, 12, 6, 128], F8, tag="wqk")
            nc.sync.dma_start(wqk_sb[:, :, :, :], wqk[:, :])
            eb_sb = P.tile([128, 12, 2, 256], BF, tag="eb")
            nc.sync.dma_start(eb_sb[:, :, :, :], eb[:, :])
            wp_sb = P.tile([128, 6, C], BF, tag="wp")
            nc.sync.dma_start(wp_sb[:, :, :], wp[:, :])
            qb_sb = P.tile([128, 6], F32, tag="qb")
            nc.sync.dma_start(qb_sb[:, :], qb[:, :])
            pbb_sb = P.tile([128, C], BF, tag="pbb")
            nc.sync.dma_start(pbb_sb[:, :], pbb[:, :])

            ones64 = P.tile([128, 64], BF, tag="ones64")
            nc.gpsimd.memset(ones64[:, :], 1.0)

            # Warmup: the PE p-state ramps to full clock only after ~3us
            # of continuous execution, and the first real matmul waits
            # ~15us for input DMA. Keep the PE busy on dummy data during
            # the wait so the v stage starts at full clock (~3us saved;
            # the dummies themselves finish right as the inputs land).
            dummy = P.tile([128, 512], BF, tag="dummy")
            nc.gpsimd.memset(dummy[:, :], 0.0)
            for w in range(23):
                wps = mm2.tile([128, 2, 512], F32, tag="mm2", name="warm")
                nc.tensor.matmul(
                    wps[:64, 0, :],
                    lhsT=ones64[:, :],
                    rhs=dummy[:, :],
                    start=True,
                    stop=True,
                )

            # M=128 so the DoubleRow denominator matmuls write partition base
            # 0 (the ISA rejects other dst bases); rows replicate the denom.
            # memset AFTER the warmup emission: the warmups gate on the
            # preceding gpsimd queue, and ones8 isn't needed until the
            # first emit_D
            ones8 = P.tile([128, 2, 128], F8, tag="ones8")
            nc.gpsimd.memset(ones8[:, :, :], 1.0)

            qkT = P.tile([128, 12, MPAD], BF, tag="qkt")
            v_sb = P.tile([128, BL, 2, C], BF, tag="v")
            AT = P.tile([128, 6, MPAD], BF, tag="at")
            # zero the pad-token tail so proj chunk 12 reads finite values
            nc.gpsimd.memset(AT[:, :, M:MPAD], 0.0)

            # ---- stage V: v[m, d] per-batch 128-token chunks ----
            # ch1 rows 69:128 hold cross-batch/pad tokens; their E rows are
            # zeroed by the exp(bias) table so they never contribute.
            for b in range(BL):
                for ch in range(2):
                    t0 = b * N + 128 * ch
                    vps = mm2.tile([128, 2, 512], F32, tag="mm2", name="vps")
                    for fi, (f0, fw) in enumerate(FCH):
                        for ct in range(6):
                            nc.tensor.matmul(
                                vps[:, fi, :fw],
                                lhsT=xT[:, ct, t0 : t0 + 128],
                                rhs=wv_sb[:, ct, f0 : f0 + fw],
                                start=(ct == 0),
                                stop=(ct == 5),
                            )
                    # ONE evac op for both f-halves: [2, 384] -> 768
                    nc.scalar.activation(
                        v_sb[:, b, ch, :], vps[:, :, 0:384], Copy
                    )

            # ---- p-loop with software pipelining ----
            # Engines execute their queues IN ORDER, so the emission order
            # IS the schedule. Per pair p, S/exp/mul runs two b-iterations
            # ahead of O/denom, and the qk GEMM groups of pair p+1 (proj
            # chunks during p=5) are interleaved between them as tensor
            # filler — the tensor queue never head-blocks on the
            # exp->mul->O dependency chain and the PE stays at full
            # p-state.

            def emit_qk_group(r, p_of_r, chunks):
                for a in qk_group_actions(r, p_of_r, chunks):
                    a()

            def qk_group_actions(r, p_of_r, chunks):
                # two m-chunks per group, cc-inner with both banks held:
                # consecutive matmuls share the same stationary weights.
                # fp8 DoubleRow: each matmul contracts TWO 128-deep c-chunks
                # (k-tiles in dim1 of both APs) at 2 fp8/partition/cycle —
                # half the bf16 stream time. q/k evac to bf16 keeps the S
                # matmul accurate; values are stored at 64x (the host scales
                # wqk by 64 to keep fp8 weights in e4m3's normal range) and
                # the exp evac divides by 64*64.
                pss = []

                def mk(cc, ci, m0, mw):
                    def t():
                        if cc == 0 and ci == 0:
                            pss[:] = [
                                mm2.tile([128, 2, 512], F32, tag="mm2", name="qkps")
                            ]
                        nc.tensor.matmul(
                            pss[0][:, ci, :mw],
                            lhsT=wqk_sb[:, r, 2 * cc : 2 * cc + 2, :],
                            rhs=x8_sb[:, 2 * cc : 2 * cc + 2, m0 : m0 + mw],
                            start=(cc == 0),
                            stop=(cc == 2),
                            perf_mode=DR,
                        )
                    return t

                def ev():
                    ps = pss[0]
                    fused = chunks[0][1] == 512 and chunks[1][1] == 512
                    if fused:  # lo groups: ONE evac op over both banks
                        srcs = [(ps[:, :, :], chunks[0][0], 1024)]
                    else:
                        srcs = [
                            (ps[:, ci, :mw], m0, mw)
                            for ci, (m0, mw) in enumerate(chunks)
                        ]
                    for src, m0, mw in srcs:
                        if r == p_of_r:  # q rows: scaled bias during evac
                            nc.scalar.activation(
                                qkT[:, r, m0 : m0 + mw],
                                src,
                                Ident,
                                bias=qb_sb[:, p_of_r : p_of_r + 1],
                            )
                        else:  # k rows: plain cast on the DVE
                            nc.vector.tensor_copy(qkT[:, r, m0 : m0 + mw], src)

                acts = [
                    mk(cc, ci, m0, mw)
                    for cc in range(3)
                    for ci, (m0, mw) in enumerate(chunks)
                ]
                acts.append(ev)
                return acts

            def qk_groups(p):
                # q tail chunk stops at M (S reads q only to 1576); k tail
                # extends to 1635 (ch1 cross-batch lhsT columns)
                for lo in (True, False):
                    for r in (p, 6 + p):
                        if lo:
                            yield (r, p, ((0, 512), (512, 512)))
                        else:
                            yield (r, p, ((1024, 512), (1536, 40 if r == p else 99)))

            def emit_S(p, b, et2, et8):
                # S^T[m, n] per head in its own bank; the odd head's lhsT
                # sits at partition base 64 (disjoint PE row groups run
                # concurrently; drains go to different banks). Then
                # exp(S^T / 4096) -> E0 (q/k are stored at 64x) and
                # E = E0 * exp(bias) on the DVE (2-byte 2x mode); the
                # table zeroes junk cols 197:256 and the ch1 pad rows
                # 69:128. Results land in half b%2 of the pair tile et2;
                # an fp8 cast (et8) feeds the DoubleRow denominators.
                bn = b * N
                bi = b % 2
                # one 2-bank tile: hj quadrants land bank-aligned so the
                # exp evacuates all four [*, 197] blocks in ONE ACT op
                psH2 = mm2.tile([128, 2, 2, 256], F32, tag="mm2", name="psH2")
                for hj in range(2):
                    hs = slice(64 * hj, 64 * (hj + 1))
                    for ch in range(2):
                        nc.tensor.matmul(
                            psH2[:, hj, ch, 0:N],
                            lhsT=qkT[hs, 6 + p, bn + 128 * ch : bn + 128 * ch + 128],
                            rhs=qkT[hs, p, bn : bn + N],
                            start=True,
                            stop=True,
                        )
                # read only the written 197-col blocks (reading bank tails
                # races with the pool's other PSUM tiles)
                nc.scalar.activation(
                    et2[:, :, :, bi, 0:197],
                    psH2[:, :, :, 0:197],
                    Exp,
                    scale=1.0 / 4096.0,
                )
                nc.vector.tensor_mul(
                    et2[:, :, :, bi, 0:197],
                    et2[:, :, :, bi, 0:197],
                    eb_sb[:, 2 * p : 2 * p + 2, :, 0:197],
                )
                # bi halves packed at stride 197 so the D matmul's moving AP
                # is exactly [K, 2, N] (the DoubleRow ISA check rejects more
                # free dims)
                nc.vector.tensor_copy(
                    et8[:, :, :, 197 * bi : 197 * bi + 197],
                    et2[:, :, :, bi, 0:197],
                )

            def emit_O(p, b, et2, psODp):
                # O^T for batch b into column half b%2 of the pair bank
                bi = b % 2
                for hj in range(2):
                    od = slice(64 * hj, 64 * (hj + 1))
                    for ch in range(2):
                        nc.tensor.matmul(
                            psODp[od, bi, 0:N],
                            lhsT=v_sb[:, b, ch, (2 * p + hj) * HD : (2 * p + hj + 1) * HD],
                            rhs=et2[:, hj, ch, bi, 0:N],
                            start=(ch == 0),
                            stop=(ch == 1),
                        )

            def emit_D(p, k, et2, et8, psODp):
                # denominators for the whole b-pair: per head, ONE fp8
                # DoubleRow matmul with the two token chunks as k-tiles and
                # the pair's two E tiles side by side in the free dim —
                # half the bf16 stream time again. The two hj's go to the
                # two banks of one mm2 tile (DoubleRow dst must sit at
                # partition base 0).
                psD = mm2.tile([128, 2, 512], F32, tag="mm2", name="psD")
                for hj in range(2):
                    nc.tensor.matmul(
                        psD[:, hj, 0 : 2 * N],
                        lhsT=ones8[:, :, :],
                        rhs=et8[:, hj, :, 0 : 2 * N],
                        start=True,
                        stop=True,
                        perf_mode=DR,
                    )
                # reciprocal_approx_fast silently corrupts at partition base
                # 64, so stage the two replicated banks into one [128, 2N]
                # tile (DVE for rows 0:64, ACT for rows 64:128) and rcp once
                # at base 0
                stage = r_pool.tile([128, 2 * N], F32, tag="stg", name="stage")
                nc.vector.tensor_copy(stage[0:64, :], psD[0:64, 0, 0 : 2 * N])
                nc.scalar.activation(
                    stage[64:128, :], psD[64:128, 1, 0 : 2 * N], Copy
                )
                rcp = r_pool.tile([128, 2, N], F32, tag="rcp")
                nc.vector.reciprocal_approx_fast(
                    out=rcp[:, :, :], in_=stage[:, :]
                )
                # ONE normalize op for the whole pair: AT cols are
                # contiguous across the two batches, psOD iterates (bi, n),
                # rcp iterates (bi, n)
                bn0 = 2 * k * N
                nc.vector.tensor_mul(
                    AT[:, p, bn0 : bn0 + 2 * N],
                    psODp[:, :, 0:N],
                    rcp[:, :, :],
                )

            def emit_proj(k):
                m0 = 128 * k
                nv = min(128, M - m0)  # valid rows (last chunk: 40)
                ob = ob_pool.tile([128, C], BF, tag="ob")
                ps = mm2.tile([128, 2, 512], F32, tag="mm2", name="pps")
                for fi, (f0, fw) in enumerate(FCH):
                    for ct in range(6):
                        nc.tensor.matmul(
                            ps[:, fi, :fw],
                            lhsT=AT[:, ct, m0 : m0 + 128],
                            rhs=wp_sb[:, ct, f0 : f0 + fw],
                            start=(ct == 0),
                            stop=(ct == 5),
                        )
                # ONE bias-add op for both f-halves
                nc.vector.tensor_add(
                    ob[:nv, :], ps[:nv, :, 0:384], pbb_sb[:nv, :]
                )
                nc.sync.dma_start(out[m0 : m0 + nv, :], ob[:nv, :])

            # proj chunk k is ready once attn(p=5, b) is done for all
            # batches its 128-token window touches
            proj_after_b = [[] for _ in range(BL)]
            for k in range(13):
                proj_after_b[min((128 * k + 127) // N, BL - 1)].append(k)

            for g in qk_groups(0):  # prologue: pair 0's projections
                emit_qk_group(*g)
            seq = [(p, b) for p in range(6) for b in range(BL)]
            gsrc = {p: iter(qk_groups(p + 1)) for p in range(5)}

            ets = {}

            def S_for(i):
                p, b = seq[i]
                pid = (p, b // 2)
                if pid not in ets:
                    ets[pid] = (
                        et_pool.tile([128, 2, 2, 2, 256], BF, tag="et", name="et2"),
                        e8_pool.tile([128, 2, 2, 512], F8, tag="et8", name="et8"),
                    )
                emit_S(p, b, *ets[pid])

            S_for(0)
            S_for(1)
            psod = {}
            for i, (p, b) in enumerate(seq):
                if i + 2 < len(seq):
                    S_for(i + 2)
                if p < 5 and b % 2 == 0:
                    for a in qk_group_actions(*next(gsrc[p])):
                        a()
                pid = (p, b // 2)
                if b % 2 == 0:
                    psod[pid] = mmo.tile([128, 2, 256], F32, tag="od", name="psODp")
                emit_O(p, b, ets[pid][0], psod[pid])
                if b % 2 == 1:
                    emit_D(p, b // 2, *ets.pop(pid), psod.pop(pid))
                    if p == 5:
                        for bb in (b - 1, b):
                            for k in proj_after_b[bb]:
                                emit_proj(k)

    nc.compile()
    return nc


def _host_prep(inputs):
    x = np.asarray(inputs["x"], np.float32)
    qkv_w = np.asarray(inputs["qkv_w"], np.float32)
    q_bias = np.asarray(inputs["q_bias"], np.float32)
    v_bias = np.asarray(inputs["v_bias"], np.float32)
    rel_table = np.asarray(inputs["rel_table"], np.float32)
    proj_w = np.asarray(inputs["proj_w"], np.float32)
    proj_b = np.asarray(inputs["proj_b"], np.float32)
    rel_index = np.asarray(inputs["rel_index"], np.int64)

    wqk_t = qkv_w[: 2 * C].T.copy()
    wqk_t[:, :C] *= SCALE  # fold q scale into weights (exact: power of 2)
    # x64 keeps the fp8 weights in e4m3's normal range (q cols would be
    # sigma~0.0025, deep in subnormals); q/k land in SBUF at 64x and the
    # exp evac divides by 64*64. [c, j*128+col] -> [kr, j, 128*ct+col]
    wqk_np = np.ascontiguousarray(
        (wqk_t * 64.0).reshape(6, 128, 12, 128).transpose(1, 2, 0, 3).reshape(128, 12 * C)
    ).astype(E4M3)
    wv_t = qkv_w[2 * C :].T
    wv_np = np.ascontiguousarray(
        wv_t.reshape(6, 128, C).transpose(1, 0, 2)
    ).astype(BF16)
    wp_t = proj_w.T
    wp_np = np.ascontiguousarray(
        wp_t.reshape(6, 128, C).transpose(1, 0, 2).reshape(128, 6 * C)
    ).astype(BF16)
    qb_np = np.ascontiguousarray((q_bias * SCALE * 64.0).reshape(6, 128).T).astype(
        np.float32
    )
    pb = (proj_b + v_bias @ proj_w.T).astype(np.float32)
    pbb_np = np.ascontiguousarray(np.tile(pb.astype(BF16)[None, :], (128, 1)))

    # exp of rel-pos bias, transposed: ebT[h, m, n] = exp(rpb[n, m, h])
    rpb = rel_table[rel_index]              # [N, N, H]
    ebT = np.exp(np.transpose(rpb, (2, 1, 0)))  # [H, m, n]
    eb_np = np.zeros((128, 12, 512), np.float32)
    for p in range(6):
        for hj in range(2):
            h = 2 * p + hj
            eb_np[0:128, 2 * p + hj, 0:N] = ebT[h, 0:128, :]
            eb_np[0:69, 2 * p + hj, 256 : 256 + N] = ebT[h, 128:N, :]
    eb_np = np.ascontiguousarray(eb_np.reshape(128, 12 * 512)).astype(BF16)

    consts = {
        "wqk": wqk_np,
        "wv": wv_np,
        "wp": wp_np,
        "eb": eb_np,
        "qb": qb_np,
        "pbb": pbb_np,
    }
    in_maps = []
    for i in range(NCORES):
        xi = x[BL * i : BL * (i + 1)].reshape(M, C)
        xpad = np.zeros((MPAD, C), np.float32)
        xpad[:M] = xi
        xt_f32 = xpad.T.reshape(6, 128, MPAD).transpose(1, 0, 2)
        xt = np.ascontiguousarray(xt_f32).astype(BF16)
        xt8 = np.ascontiguousarray(xt_f32).astype(E4M3)
        in_maps.append({"x": xt, "x8": xt8, **consts})
    return in_maps


def _run(inputs, trace=False):
    global _NC
    if _NC is None:
        _NC = _build()
    in_maps = _host_prep(inputs)
    res = run_bass_kernel_spmd(_NC, in_maps, core_ids=list(range(NCORES)), trace=trace)
    outs = [
        np.asarray(res.results[i]["out"]).astype(np.float32).reshape(BL, N, C)
        for i in range(NCORES)
    ]
    full = np.concatenate(outs, axis=0)
    return full, res


def kernel(**inputs) -> np.ndarray:
    full, _ = _run(inputs, trace=False)
    return full



# revision 41
# speedup vs baseline: 1.0378x; 1.0378x over previous
"""BEiT-style attention (B=64, N=197, C=768, H=12, rel-pos bias) on 8 TRN2 cores.

Data-parallel over batch: 8 batch items per core, no collectives.

Design notes (what made this fast — 232us baseline -> ~166us):
  - Engines execute their queues IN ORDER, so emission order IS the
    schedule, and the TRN2 PE has p-states (full 2.4 GHz only after ~3us
    of continuous execution; a stall drops it to 1.2 GHz). The whole
    kernel is one software-pipelined stream in which the tensor engine
    never head-blocks: per head-pair p, S/exp runs two b-iterations
    ahead of O/denom, and the qk GEMM of pair p+1 (proj chunks during
    p=5) is emitted block-wise between them as tensor filler that covers
    the exp->mul->O latency.
  - Rel-pos bias enters as a multiplicative exp(bias) table applied to
    exp(S) on the DVE (2-byte 2x mode) instead of an identity-matmul
    PSUM prefill; softmax denominators use ones-matmuls batched over
    b-pairs, and the ~5x-faster reciprocal_approx_fast custom DVE op.
  - All attention matmuls use K=128 (token chunks padded cross-batch;
    the exp(bias) table zeroes the pad rows so they contribute nothing).
  - Projection runs over 13 flat 128-token chunks (MPAD = 1664 = 13*128);
    proj bias is added by the DVE during PSUM evacuation (no ones-row
    matmul). v_bias is folded into the proj bias on the host (softmax
    rows sum to 1); q_bias/scale fold into the qk weights / ACT evac.
  - Every DRAM input is host-prearranged to its exact SBUF layout; the
    wv/x loads are split so the first v matmul starts ~5us earlier, and
    ~23 warmup matmuls on dummy data keep the PE busy (and its p-state
    ramped) during the input-DMA wait — the v stage then starts at full
    clock with all inputs resident.
    (Empirically, finer DMA splitting [3+ pieces of a hot tile] or
    fine-grained interleaving of accumulation groups makes ALL matmuls
    ~20% slower — keep hot tiles to <=2 DMA pieces and keep matmul
    accumulation groups contiguous.)
  - fp8 e4m3 DoubleRow matmuls (2 fp8/partition/cycle, two 128-deep
    k-tiles per instruction) halve the q/k GEMM and the softmax
    denominator matmuls. q/k evacuate to bf16 (fp8 storage would blow
    the error budget: 2.1e-2 > 2e-2); weights carry a 64x scale so the
    fp8 values sit in e4m3's normal range, undone in the exp evac
    (scale=1/4096). Denominators read an fp8 cast of E (one extra DVE
    copy per (p,b)); the quantization averages out over 197 summands.
    v/proj/O GEMMs must stay bf16 — their error hits the output 1:1,
    and two-sided fp8 costs ~2.6%: over budget. Expected ~1.66e-2.
"""

import numpy as np
import ml_dtypes

import concourse.bass as bass
import concourse.mybir as mybir
import concourse.tile as tile
from concourse import bacc
from concourse.bass_utils import run_bass_kernel_spmd

BF16 = ml_dtypes.bfloat16
E4M3 = ml_dtypes.float8_e4m3
F32 = mybir.dt.float32
BF = mybir.dt.bfloat16
F8 = mybir.dt.float8e4
DR = mybir.MatmulPerfMode.DoubleRow

B, N, C = 64, 197, 768
H, HD = 12, 64
NCORES = 8
BL = B // NCORES            # 8 batches per core
M = BL * N                  # 1576 real tokens per core
MPAD = 1664                 # 13 * 128
SCALE = HD ** -0.5
MCH = [(0, 512), (512, 512), (1024, 512), (1536, 128)]
# v: two equal f-halves so the PSUM pair evacuates as ONE rectangular
# [2, 384] op — the side engines are op-count-bound (~165-400ns fixed
# cost per instruction regardless of size). proj keeps 1-bank chunks.
FCH = [(0, 384), (384, 384)]
PFCH = [(0, 512), (512, 256)]

_NC = None


def _build():
    nc = bacc.Bacc("TRN2", target_bir_lowering=False, debug=False)

    x = nc.dram_tensor("x", [128, 6, MPAD], BF, kind="ExternalInput")
    x8 = nc.dram_tensor("x8", [128, 6, MPAD], F8, kind="ExternalInput")
    wqk = nc.dram_tensor("wqk", [128, 12 * C], F8, kind="ExternalInput")
    wv = nc.dram_tensor("wv", [128, 6, C], BF, kind="ExternalInput")
    wp = nc.dram_tensor("wp", [128, 6 * C], BF, kind="ExternalInput")
    eb = nc.dram_tensor("eb", [128, 12 * 512], BF, kind="ExternalInput")
    qb = nc.dram_tensor("qb", [128, 6], F32, kind="ExternalInput")
    pbb = nc.dram_tensor("pbb", [128, C], BF, kind="ExternalInput")
    out = nc.dram_tensor("out", [M, C], BF, kind="ExternalOutput")

    Ident = mybir.ActivationFunctionType.Identity
    Copy = mybir.ActivationFunctionType.Copy
    Exp = mybir.ActivationFunctionType.Exp

    with tile.TileContext(nc) as tc:
        with (
            tc.tile_pool(name="persist", bufs=1) as P,
            tc.tile_pool(name="et", bufs=4) as et_pool,
            tc.tile_pool(name="e8", bufs=4) as e8_pool,
            tc.tile_pool(name="rcp", bufs=4) as r_pool,
            tc.tile_pool(name="ob", bufs=3) as ob_pool,
            # PSUM = 16KB/partition = 8 banks. A dedicated ring of 2
            # two-bank (4KB) tiles for psH2 (the S->exp pipeline; the v
            # stage borrows it before the p-loop starts) + a ring of 4
            # one-bank tiles for qk/psOD/psD/proj/warmups: 2*4 + 4*2 =
            # 16KB exactly. A single shared ring serializes the pipeline
            # (measured +10us of PE gaps).
            tc.tile_pool(name="mm2", bufs=2, space="PSUM") as mm2,
            tc.tile_pool(name="mm", bufs=4, space="PSUM") as mm,
        ):
            # ---- inputs to SBUF (one DMA per tensor, pre-laid-out) ----
            # wv/x split by first use; more than 2 pieces per hot tile
            # measurably slows every matmul down (see module docstring).
            wv_sb = P.tile([128, 6, C], BF, tag="wv")
            xT = P.tile([128, 6, MPAD], BF, tag="xt")
            # staged by first use: the first v matmuls (batch 0, f-half 0)
            # need only wv cols 0:512 and x tokens 0:256; the 640 split
            # point buys ~9.6us of v compute before the x tail is needed
            # fused v evac needs both f-halves at the first group, so wv
            # loads in one piece before the x head
            nc.sync.dma_start(wv_sb[:, :, :], wv[:, :, :])
            nc.sync.dma_start(xT[:, :, 0:640], x[:, :, 0:640])
            nc.sync.dma_start(xT[:, :, 640:MPAD], x[:, :, 640:MPAD])
            x8_sb = P.tile([128, 6, MPAD], F8, tag="x8")
            nc.sync.dma_start(x8_sb[:, :, :], x8[:, :, :])
            wqk_sb = P.tile([128, 12, 6, 128], F8, tag="wqk")
            nc.sync.dma_start(wqk_sb[:, :, :, :], wqk[:, :])
            eb_sb = P.tile([128, 12, 2, 256], BF, tag="eb")
            nc.sync.dma_start(eb_sb[:, :, :, :], eb[:, :])
            wp_sb = P.tile([128, 6, C], BF, tag="wp")
            nc.sync.dma_start(wp_sb[:, :, :], wp[:, :])
            qb_sb = P.tile([128, 6], F32, tag="qb")
            nc.sync.dma_start(qb_sb[:, :], qb[:, :])
            pbb_sb = P.tile([128, C], BF, tag="pbb")
            nc.sync.dma_start(pbb_sb[:, :], pbb[:, :])

            ones64 = P.tile([128, 64], BF, tag="ones64")
            nc.gpsimd.memset(ones64[:, :], 1.0)

            # Warmup: the PE p-state ramps to full clock only after ~3us
            # of continuous execution, and the first real matmul waits
            # ~15us for input DMA. Keep the PE busy on dummy data during
            # the wait so the v stage starts at full clock (~3us saved;
            # the dummies themselves finish right as the inputs land).
            dummy = P.tile([128, 512], BF, tag="dummy")
            nc.gpsimd.memset(dummy[:, :], 0.0)
            for w in range(23):
                wps = mm.tile([128, 512], F32, tag="mm", name="warm")
                nc.tensor.matmul(
                    wps[:64, :],
                    lhsT=ones64[:, :],
                    rhs=dummy[:, :],
                    start=True,
                    stop=True,
                )

            # M=128 so the DoubleRow denominator matmuls write partition base
            # 0 (the ISA rejects other dst bases); rows replicate the denom.
            # memset AFTER the warmup emission: the warmups gate on the
            # preceding gpsimd queue, and ones8 isn't needed until the
            # first emit_D
            ones8 = P.tile([128, 2, 128], F8, tag="ones8")
            nc.gpsimd.memset(ones8[:, :, :], 1.0)

            qkT = P.tile([128, 12, MPAD], BF, tag="qkt")
            v_sb = P.tile([128, BL, 2, C], BF, tag="v")
            AT = P.tile([128, 6, MPAD], BF, tag="at")
            # zero the pad-token tail so proj chunk 12 reads finite values
            nc.gpsimd.memset(AT[:, :, M:MPAD], 0.0)

            # ---- stage V: v[m, d] per-batch 128-token chunks ----
            # ch1 rows 69:128 hold cross-batch/pad tokens; their E rows are
            # zeroed by the exp(bias) table so they never contribute.
            for b in range(BL):
                for ch in range(2):
                    t0 = b * N + 128 * ch
                    vps = mm2.tile([128, 2, 512], F32, tag="mm2", name="vps")
                    for fi, (f0, fw) in enumerate(FCH):
                        for ct in range(6):
                            nc.tensor.matmul(
                                vps[:, fi, :fw],
                                lhsT=xT[:, ct, t0 : t0 + 128],
                                rhs=wv_sb[:, ct, f0 : f0 + fw],
                                start=(ct == 0),
                                stop=(ct == 5),
                            )
                    # ONE evac op for both f-halves: [2, 384] -> 768
                    nc.scalar.activation(
                        v_sb[:, b, ch, :], vps[:, :, 0:384], Copy
                    )

            # ---- p-loop with software pipelining ----
            # Engines execute their queues IN ORDER, so the emission order
            # IS the schedule. Per pair p, S/exp/mul runs two b-iterations
            # ahead of O/denom, and the qk GEMM groups of pair p+1 (proj
            # chunks during p=5) are interleaved between them as tensor
            # filler — the tensor queue never head-blocks on the
            # exp->mul->O dependency chain and the PE stays at full
            # p-state.

            def emit_qk_group(r, p_of_r, chunks):
                for a in qk_group_actions(r, p_of_r, chunks):
                    a()

            def qk_group_actions(r, p_of_r, chunks):
                # two m-chunks per group, cc-inner with both banks held:
                # consecutive matmuls share the same stationary weights.
                # fp8 DoubleRow: each matmul contracts TWO 128-deep c-chunks
                # (k-tiles in dim1 of both APs) at 2 fp8/partition/cycle —
                # half the bf16 stream time. q/k evac to bf16 keeps the S
                # matmul accurate; values are stored at 64x (the host scales
                # wqk by 64 to keep fp8 weights in e4m3's normal range) and
                # the exp evac divides by 64*64.
                pss = []

                def mk(cc, ci, m0, mw):
                    def t():
                        if cc == 0 and ci == 0:
                            pss[:] = [
                                mm.tile([128, 512], F32, tag="mm", name=f"qkps{i}")
                                for i in range(len(chunks))
                            ]
                        nc.tensor.matmul(
                            pss[ci][:, :mw],
                            lhsT=wqk_sb[:, r, 2 * cc : 2 * cc + 2, :],
                            rhs=x8_sb[:, 2 * cc : 2 * cc + 2, m0 : m0 + mw],
                            start=(cc == 0),
                            stop=(cc == 2),
                            perf_mode=DR,
                        )
                    return t

                def ev():
                    for ps, (m0, mw) in zip(pss, chunks):
                        if r == p_of_r:  # q rows: scaled bias during evac
                            nc.scalar.activation(
                                qkT[:, r, m0 : m0 + mw],
                                ps[:, :mw],
                                Ident,
                                bias=qb_sb[:, p_of_r : p_of_r + 1],
                            )
                        else:  # k rows: plain cast on the DVE
                            nc.vector.tensor_copy(
                                qkT[:, r, m0 : m0 + mw], ps[:, :mw]
                            )

                acts = [
                    mk(cc, ci, m0, mw)
                    for cc in range(3)
                    for ci, (m0, mw) in enumerate(chunks)
                ]
                acts.append(ev)
                return acts

            def qk_groups(p):
                # q tail chunk stops at M (S reads q only to 1576); k tail
                # extends to 1635 (ch1 cross-batch lhsT columns)
                for lo in (True, False):
                    for r in (p, 6 + p):
                        if lo:
                            yield (r, p, ((0, 512), (512, 512)))
                        else:
                            yield (r, p, ((1024, 512), (1536, 40 if r == p else 99)))

            def emit_S(p, b, et2, et8):
                # S^T[m, n] per head in its own bank; the odd head's lhsT
                # sits at partition base 64 (disjoint PE row groups run
                # concurrently; drains go to different banks). Then
                # exp(S^T / 4096) -> E0 (q/k are stored at 64x) and
                # E = E0 * exp(bias) on the DVE (2-byte 2x mode); the
                # table zeroes junk cols 197:256 and the ch1 pad rows
                # 69:128. Results land in half b%2 of the pair tile et2;
                # an fp8 cast (et8) feeds the DoubleRow denominators.
                bn = b * N
                bi = b % 2
                # one 2-bank tile: hj quadrants land bank-aligned so the
                # exp evacuates all four [*, 197] blocks in ONE ACT op
                psH2 = mm2.tile([128, 2, 2, 256], F32, tag="mm2", name="psH2")
                for hj in range(2):
                    hs = slice(64 * hj, 64 * (hj + 1))
                    for ch in range(2):
                        nc.tensor.matmul(
                            psH2[:, hj, ch, 0:N],
                            lhsT=qkT[hs, 6 + p, bn + 128 * ch : bn + 128 * ch + 128],
                            rhs=qkT[hs, p, bn : bn + N],
                            start=True,
                            stop=True,
                        )
                # read only the written 197-col blocks (reading bank tails
                # races with the pool's other PSUM tiles)
                nc.scalar.activation(
                    et2[:, :, :, bi, 0:197],
                    psH2[:, :, :, 0:197],
                    Exp,
                    scale=1.0 / 4096.0,
                )
                nc.vector.tensor_mul(
                    et2[:, :, :, bi, 0:197],
                    et2[:, :, :, bi, 0:197],
                    eb_sb[:, 2 * p : 2 * p + 2, :, 0:197],
                )
                # bi halves packed at stride 197 so the D matmul's moving AP
                # is exactly [K, 2, N] (the DoubleRow ISA check rejects more
                # free dims)
                nc.vector.tensor_copy(
                    et8[:, :, :, 197 * bi : 197 * bi + 197],
                    et2[:, :, :, bi, 0:197],
                )

            def emit_O(p, b, et2, psODp):
                # O^T for batch b into column half b%2 of the pair bank
                bi = b % 2
                for hj in range(2):
                    od = slice(64 * hj, 64 * (hj + 1))
                    for ch in range(2):
                        nc.tensor.matmul(
                            psODp[od, bi, 0:N],
                            lhsT=v_sb[:, b, ch, (2 * p + hj) * HD : (2 * p + hj + 1) * HD],
                            rhs=et2[:, hj, ch, bi, 0:N],
                            start=(ch == 0),
                            stop=(ch == 1),
                        )

            def emit_D(p, k, et2, et8, psODp):
                # denominators for the whole b-pair: per head, ONE fp8
                # DoubleRow matmul with the two token chunks as k-tiles and
                # the pair's two E tiles side by side in the free dim —
                # half the bf16 stream time again. The two hj's go to the
                # two banks of one mm2 tile (DoubleRow dst must sit at
                # partition base 0).
                psD = [
                    mm.tile([128, 512], F32, tag="mm", name=f"psD{hj}")
                    for hj in range(2)
                ]
                for hj in range(2):
                    nc.tensor.matmul(
                        psD[hj][:, 0 : 2 * N],
                        lhsT=ones8[:, :, :],
                        rhs=et8[:, hj, :, 0 : 2 * N],
                        start=True,
                        stop=True,
                        perf_mode=DR,
                    )
                # reciprocal_approx_fast silently corrupts at partition base
                # 64, so stage the two replicated banks into one [128, 2N]
                # tile (DVE for rows 0:64, ACT for rows 64:128) and rcp once
                # at base 0
                stage = r_pool.tile([128, 2 * N], F32, tag="stg", name="stage")
                nc.vector.tensor_copy(stage[0:64, :], psD[0][0:64, 0 : 2 * N])
                nc.scalar.activation(
                    stage[64:128, :], psD[1][64:128, 0 : 2 * N], Copy
                )
                rcp = r_pool.tile([128, 2, N], F32, tag="rcp")
                nc.vector.reciprocal_approx_fast(
                    out=rcp[:, :, :], in_=stage[:, :]
                )
                # ONE normalize op for the whole pair: AT cols are
                # contiguous across the two batches, psOD iterates (bi, n),
                # rcp iterates (bi, n)
                bn0 = 2 * k * N
                nc.vector.tensor_mul(
                    AT[:, p, bn0 : bn0 + 2 * N],
                    psODp[:, :, 0:N],
                    rcp[:, :, :],
                )

            def emit_proj(k):
                m0 = 128 * k
                nv = min(128, M - m0)  # valid rows (last chunk: 40)
                ob = ob_pool.tile([128, C], BF, tag="ob")
                for f0, fw in PFCH:
                    ps = mm.tile([128, 512], F32, tag="mm", name="pps")
                    for ct in range(6):
                        nc.tensor.matmul(
                            ps[:, :fw],
                            lhsT=AT[:, ct, m0 : m0 + 128],
                            rhs=wp_sb[:, ct, f0 : f0 + fw],
                            start=(ct == 0),
                            stop=(ct == 5),
                        )
                    nc.vector.tensor_add(
                        ob[:nv, f0 : f0 + fw],
                        ps[:nv, :fw],
                        pbb_sb[:nv, f0 : f0 + fw],
                    )
                nc.sync.dma_start(out[m0 : m0 + nv, :], ob[:nv, :])

            # proj chunk k is ready once attn(p=5, b) is done for all
            # batches its 128-token window touches
            proj_after_b = [[] for _ in range(BL)]
            for k in range(13):
                proj_after_b[min((128 * k + 127) // N, BL - 1)].append(k)

            for g in qk_groups(0):  # prologue: pair 0's projections
                emit_qk_group(*g)
            seq = [(p, b) for p in range(6) for b in range(BL)]
            gsrc = {p: iter(qk_groups(p + 1)) for p in range(5)}

            ets = {}

            def S_for(i):
                p, b = seq[i]
                pid = (p, b // 2)
                if pid not in ets:
                    ets[pid] = (
                        et_pool.tile([128, 2, 2, 2, 256], BF, tag="et", name="et2"),
                        e8_pool.tile([128, 2, 2, 512], F8, tag="et8", name="et8"),
                    )
                emit_S(p, b, *ets[pid])

            S_for(0)
            S_for(1)
            psod = {}
            for i, (p, b) in enumerate(seq):
                if i + 2 < len(seq):
                    S_for(i + 2)
                if p < 5 and b % 2 == 0:
                    for a in qk_group_actions(*next(gsrc[p])):
                        a()
                pid = (p, b // 2)
                if b % 2 == 0:
                    psod[pid] = mm.tile([128, 2, 256], F32, tag="mm", name="psODp")
                emit_O(p, b, ets[pid][0], psod[pid])
                if b % 2 == 1:
                    emit_D(p, b // 2, *ets.pop(pid), psod.pop(pid))
                    if p == 5:
                        for bb in (b - 1, b):
                            for k in proj_after_b[bb]:
                                emit_proj(k)

    nc.compile()
    return nc


def _host_prep(inputs):
    x = np.asarray(inputs["x"], np.float32)
    qkv_w = np.asarray(inputs["qkv_w"], np.float32)
    q_bias = np.asarray(inputs["q_bias"], np.float32)
    v_bias = np.asarray(inputs["v_bias"], np.float32)
    rel_table = np.asarray(inputs["rel_table"], np.float32)
    proj_w = np.asarray(inputs["proj_w"], np.float32)
    proj_b = np.asarray(inputs["proj_b"], np.float32)
    rel_index = np.asarray(inputs["rel_index"], np.int64)

    wqk_t = qkv_w[: 2 * C].T.copy()
    wqk_t[:, :C] *= SCALE  # fold q scale into weights (exact: power of 2)
    # x64 keeps the fp8 weights in e4m3's normal range (q cols would be
    # sigma~0.0025, deep in subnormals); q/k land in SBUF at 64x and the
    # exp evac divides by 64*64. [c, j*128+col] -> [kr, j, 128*ct+col]
    wqk_np = np.ascontiguousarray(
        (wqk_t * 64.0).reshape(6, 128, 12, 128).transpose(1, 2, 0, 3).reshape(128, 12 * C)
    ).astype(E4M3)
    wv_t = qkv_w[2 * C :].T
    wv_np = np.ascontiguousarray(
        wv_t.reshape(6, 128, C).transpose(1, 0, 2)
    ).astype(BF16)
    wp_t = proj_w.T
    wp_np = np.ascontiguousarray(
        wp_t.reshape(6, 128, C).transpose(1, 0, 2).reshape(128, 6 * C)
    ).astype(BF16)
    qb_np = np.ascontiguousarray((q_bias * SCALE * 64.0).reshape(6, 128).T).astype(
        np.float32
    )
    pb = (proj_b + v_bias @ proj_w.T).astype(np.float32)
    pbb_np = np.ascontiguousarray(np.tile(pb.astype(BF16)[None, :], (128, 1)))

    # exp of rel-pos bias, transposed: ebT[h, m, n] = exp(rpb[n, m, h])
    rpb = rel_table[rel_index]              # [N, N, H]
    ebT = np.exp(np.transpose(rpb, (2, 1, 0)))  # [H, m, n]
    eb_np = np.zeros((128, 12, 512), np.float32)
    for p in range(6):
        for hj in range(2):
            h = 2 * p + hj
            eb_np[0:128, 2 * p + hj, 0:N] = ebT[h, 0:128, :]
            eb_np[0:69, 2 * p + hj, 256 : 256 + N] = ebT[h, 128:N, :]
    eb_np = np.ascontiguousarray(eb_np.reshape(128, 12 * 512)).astype(BF16)

    consts = {
        "wqk": wqk_np,
        "wv": wv_np,
        "wp": wp_np,
        "eb": eb_np,
        "qb": qb_np,
        "pbb": pbb_np,
    }
    in_maps = []
    for i in range(NCORES):
        xi = x[BL * i : BL * (i + 1)].reshape(M, C)
        xpad = np.zeros((MPAD, C), np.float32)
        xpad[:M] = xi
        xt_f32 = xpad.T.reshape(6, 128, MPAD).transpose(1, 0, 2)
        xt = np.ascontiguousarray(xt_f32).astype(BF16)
        xt8 = np.ascontiguousarray(xt_f32).astype(E4M3)
        in_maps.append({"x": xt, "x8": xt8, **consts})
    return in_maps


def _run(inputs, trace=False):
    global _NC
    if _NC is None:
        _NC = _build()
    in_maps = _host_prep(inputs)
    res = run_bass_kernel_spmd(_NC, in_maps, core_ids=list(range(NCORES)), trace=trace)
    outs = [
        np.asarray(res.results[i]["out"]).astype(np.float32).reshape(BL, N, C)
        for i in range(NCORES)
    ]
    full = np.concatenate(outs, axis=0)
    return full, res


def kernel(**inputs) -> np.ndarray:
    full, _ = _run(inputs, trace=False)
    return full

